# revision 2
# baseline (speedup 1.0000x reference)
"""Causal GQA self-attention on 8 Trainium2 NeuronCores.

Problem: B=2, S=2048, HIDDEN=2048, 16 q-heads, 4 kv-heads, head_dim=128, fp32.

Sharding: core c = 4*b + g  (b = batch, g = head-group).
Each core owns batch b and q-heads [4g, 4g+4) plus their shared kv-head g
(GQA maps q-head h -> kv-head h//4, so group g exactly owns kv-head g).

Per-core pipeline (everything in transposed [feature, seq] layout, bf16
matmul operands / fp32 PSUM accumulation):
  1. Projections: QT[d,s] / KT[d,s] / VT[d,s] = W*.T @ X.T  (X.T shipped from
     host pre-cast to bf16).  V[s,d] obtained by PE-transposing VT.
  2. Attention per 512-wide query chunk, per head: for each 128-wide key tile j,
     ST = KT_j.T @ QT_chunk  ->  P = exp(ST/sqrt(d)) (ACT, fused scale)
     -> causal mask multiply on diagonal tiles (DVE)
     -> attnT += V_j.T @ P (PE, PSUM accum).
     Row-sums l are NOT computed on the PE per tile: P tiles are accumulated
     into sumP on DVE (even heads) / GpSimd (odd heads), and a single
     ones.T @ sumP matmul per (chunk, head) yields l.  The QK matmul for
     key tile j+2 is emitted before the PV matmul of tile j so the PE never
     waits on the ACT exp.
  3. AllGather attnT chunk across the 4 cores of the batch (f16) -> full
     [2048, 512] attnT; O-projection with this core's 512 Wo columns ->
     outT slice.  O-projection of chunk c is emitted after attention of
     chunk c+2 so the AllGather latency hides under attention compute.
Host gathers: out[b][:, 512g:512(g+1)] = core(b,g) outT.T.

bf16 operands keep the PE at 1 cycle/column and enable Fast Weight Load
(fp32 stationaries disable FWL and run the array in its max-power mode,
which triggers sustained HAM/SW clock throttling).
"""

import numpy as np

HID = 2048
S = 2048
B = 2
NH = 16          # q heads total
D = 128          # head dim
G = 4            # head groups == cores per batch
HPG = NH // G    # q heads per group (4)
CH = 512         # seq chunk (free dim of moving operands)
NCH = S // CH    # 4 chunks
NKT = S // 128   # 16 key tiles
SCALE = 1.0 / float(np.sqrt(D))
OPROJ_LAG = 2    # chunks of attention emitted before each O-projection

_CACHED_NC = None


def _build_nc(sim_mode=False, reps=1):
    import concourse.mybir as mybir
    import concourse.tile as tile
    from concourse import bacc

    F32 = mybir.dt.float32
    BF16 = mybir.dt.bfloat16
    F16 = mybir.dt.float16
    Copy = mybir.ActivationFunctionType.Copy
    Exp = mybir.ActivationFunctionType.Exp

    nc = bacc.Bacc("TRN2", target_bir_lowering=False, debug=False,
                   num_devices=1 if sim_mode else 8)

    # ---- per-core input shards ----
    xt = nc.declare_dram_parameter("xt", [HID, S], BF16, isOutput=False)
    wq = nc.declare_dram_parameter("wq", [HID, HPG * D], BF16, isOutput=False)
    wk = nc.declare_dram_parameter("wk", [HID, D], BF16, isOutput=False)
    wv = nc.declare_dram_parameter("wv", [HID, D], BF16, isOutput=False)
    wo = nc.declare_dram_parameter("wo", [HID, CH], F16, isOutput=False)
    bq = nc.declare_dram_parameter("bq", [D, HPG], F32, isOutput=False)
    bk = nc.declare_dram_parameter("bk", [D, 1], F32, isOutput=False)
    bv = nc.declare_dram_parameter("bv", [D, 1], F32, isOutput=False)
    bo = nc.declare_dram_parameter("bo", [D, HPG], F32, isOutput=False)
    masks = nc.declare_dram_parameter("masks", [128, 128], BF16, isOutput=False)
    keybias = nc.declare_dram_parameter("keybias", [128, NKT], F32, isOutput=False)
    ident = nc.declare_dram_parameter("ident", [128, 128], BF16, isOutput=False)
    ones = nc.declare_dram_parameter("ones", [128, 1], BF16, isOutput=False)
    out = nc.declare_dram_parameter("out", [CH, S], F32, isOutput=True)

    groups = [[0, 1, 2, 3], [4, 5, 6, 7]]

    with tile.TileContext(nc) as tc:
        for _rep in range(reps):
            with (
                tc.tile_pool(name="persist", bufs=1) as persist,
                tc.tile_pool(name="dram", bufs=4, space="DRAM") as dram,
            ):
                # ---- persistent SBUF state ----
                qt_sb = persist.tile([128, HPG, S], BF16)      # QT per head  [d, h, s]
                kt_sb = persist.tile([128, S], BF16)           # KT           [d, s]
                v_sb = persist.tile([128, NKT, D], BF16)       # V            [s, j, d]
                masks_sb = persist.tile([128, 128], BF16)
                kb_sb = persist.tile([128, NKT], F32)
                ones_sb = persist.tile([128, 1], BF16)
                bq_sb = persist.tile([D, HPG], F32)
                bk_sb = persist.tile([D, 1], F32)
                bv_sb = persist.tile([D, 1], F32)
                bo_sb = persist.tile([D, HPG], F32)
                wo_sb = persist.tile([128, NKT, CH], F16)

                nc.sync.dma_start(out=masks_sb, in_=masks.ap())
                nc.sync.dma_start(out=kb_sb, in_=keybias.ap())
                nc.sync.dma_start(out=ones_sb, in_=ones.ap())
                nc.sync.dma_start(out=bq_sb, in_=bq.ap())
                nc.sync.dma_start(out=bk_sb, in_=bk.ap())
                nc.sync.dma_start(out=bv_sb, in_=bv.ap())
                nc.sync.dma_start(out=bo_sb, in_=bo.ap())

                # ================= Phase A: projections =================
                with (
                    tc.tile_pool(name="wA", bufs=1) as wA,
                    tc.tile_pool(name="xs", bufs=6) as xs,
                    tc.tile_pool(name="vts", bufs=2) as vts,
                    tc.tile_pool(name="psA", bufs=1, space="PSUM") as psA,
                    tc.tile_pool(name="psT", bufs=1, space="PSUM") as psT,
                ):
                    idr_sb = wA.tile([128, 128], BF16)
                    nc.gpsimd.dma_start(out=idr_sb, in_=ident.ap())
                    wq_sb = wA.tile([128, NKT, HPG * D], BF16)
                    wq_r = wq.ap().rearrange("(t p) n -> p t n", p=128)
                    nc.gpsimd.dma_start(out=wq_sb[:, :4, :], in_=wq_r[:, :4, :])
                    wk_sb = wA.tile([128, NKT, D], BF16)
                    nc.gpsimd.dma_start(
                        out=wk_sb, in_=wk.ap().rearrange("(t p) n -> p t n", p=128))
                    wv_sb = wA.tile([128, NKT, D], BF16)
                    nc.gpsimd.dma_start(
                        out=wv_sb, in_=wv.ap().rearrange("(t p) n -> p t n", p=128))
                    nc.gpsimd.dma_start(out=wq_sb[:, 4:8, :], in_=wq_r[:, 4:8, :])
                    nc.gpsimd.dma_start(out=wq_sb[:, 8:12, :], in_=wq_r[:, 8:12, :])
                    nc.gpsimd.dma_start(out=wq_sb[:, 12:, :], in_=wq_r[:, 12:, :])

                    for c in range(NCH):
                        sq = slice(c * CH, (c + 1) * CH)
                        ps_q = psA.tile([128, HPG, CH], F32, name="ps_q")  # 4 banks
                        ps_k = psA.tile([128, CH], F32, name="ps_k", bufs=2)
                        ps_v = psA.tile([128, CH], F32, name="ps_v")
                        for t in range(NKT):
                            xt_t = xs.tile([128, CH], BF16, name="xt_t")
                            nc.gpsimd.dma_start(
                                out=xt_t, in_=xt[t * 128:(t + 1) * 128, sq])
                            st, sp = (t == 0), (t == NKT - 1)
                            for h in range(HPG):
                                nc.tensor.matmul(
                                    ps_q[:, h, :],
                                    lhsT=wq_sb[:, t, h * D:(h + 1) * D],
                                    rhs=xt_t, start=st, stop=sp)
                            nc.tensor.matmul(ps_k, lhsT=wk_sb[:, t, :], rhs=xt_t,
                                             start=st, stop=sp)
                            nc.tensor.matmul(ps_v, lhsT=wv_sb[:, t, :], rhs=xt_t,
                                             start=st, stop=sp)
                        for h in range(HPG):
                            nc.vector.tensor_scalar_add(qt_sb[:, h, sq],
                                                        ps_q[:, h, :],
                                                        bq_sb[:, h:h + 1])
                        nc.vector.tensor_scalar_add(kt_sb[:, sq], ps_k, bk_sb)
                        vt_t = vts.tile([128, CH], BF16, name="vt_t")
                        nc.vector.tensor_scalar_add(vt_t, ps_v, bv_sb)
                        for u in range(4):
                            j = 4 * c + u
                            ps_tp = psT.tile([128, 128], BF16, name="ps_tp")
                            nc.tensor.transpose(
                                ps_tp, vt_t[:, u * 128:(u + 1) * 128], idr_sb)
                            nc.scalar.activation(v_sb[:, j, :], ps_tp, Copy)
                        if c == 0:
                            nc.gpsimd.dma_start(
                                out=wo_sb,
                                in_=wo.ap().rearrange("(t p) n -> p t n", p=128))

                # ================= Phase B: attention + O-projection =================
                with (
                    tc.tile_pool(name="ps_s", bufs=3, space="PSUM") as ps_s_pool,
                    tc.tile_pool(name="ps_pv", bufs=2, space="PSUM") as ps_pv_pool,
                    tc.tile_pool(name="ps_l", bufs=1, space="PSUM") as ps_l_pool,
                    tc.tile_pool(name="ps_o", bufs=2, space="PSUM") as ps_o_pool,
                    tc.tile_pool(name="ps_p", bufs=4) as pp,
                    tc.tile_pool(name="sump", bufs=2) as sump,
                    tc.tile_pool(name="att", bufs=2) as att,
                    tc.tile_pool(name="rbp", bufs=2) as rbp,
                    tc.tile_pool(name="mo", bufs=2) as mo,
                    tc.tile_pool(name="ost", bufs=2) as ost,
                ):
                    def oproj(c, ag_out):
                        sq = slice(c * CH, (c + 1) * CH)
                        m_all = mo.tile([128, NKT, CH], F16, name="m_all")
                        ag_r = ag_out.rearrange("g h p n -> p (g h) n")
                        for ct in range(NKT):
                            nc.sync.dma_start(out=m_all[:, ct, :],
                                              in_=ag_r[:, ct, :])
                        for t in range(HPG):
                            ps_o = ps_o_pool.tile([128, CH], F32, name="ps_o")
                            for ct in range(NKT):
                                nc.tensor.matmul(
                                    ps_o, lhsT=wo_sb[:, ct, t * 128:(t + 1) * 128],
                                    rhs=m_all[:, ct, :],
                                    start=(ct == 0), stop=(ct == NKT - 1))
                            o_sb = ost.tile([128, CH], F32, name="o_sb")
                            nc.vector.tensor_scalar_add(o_sb, ps_o,
                                                        bo_sb[:, t:t + 1])
                            nc.sync.dma_start(
                                out=out[t * 128:(t + 1) * 128, sq], in_=o_sb)

                    pending = []
                    for c in range(NCH):
                        njt = 4 * c + 4  # causal: key tiles 0..4c+3
                        ag_in = dram.tile([HPG, 128, CH], F16, name="ag_in")
                        for h in range(HPG):
                            # even heads accumulate P row-sums on DVE, odd on
                            # GpSimd (tensor_tensor never contends for the
                            # shared SBUF port pair)
                            eng = nc.vector if h % 2 == 0 else nc.gpsimd
                            ps_pv = ps_pv_pool.tile([128, CH], F32, name="ps_pv")
                            sum_p = sump.tile([128, CH], F32, name="sum_p")
                            sum_pb = sump.tile([128, CH], BF16, name="sum_pb")
                            ps_s = {}
                            p_sb = {}

                            def qk(j):
                                # causal: key tile j only reaches queries
                                # >= 128*r into the chunk (r = j - 4c >= 0 on
                                # the diagonal); restrict to that suffix.
                                r = max(0, j - 4 * c)
                                cs = slice(128 * r, CH)
                                qs_ = slice(c * CH + 128 * r, (c + 1) * CH)
                                ps_s[j] = ps_s_pool.tile([128, CH], F32,
                                                         name="ps_s")
                                nc.tensor.matmul(
                                    ps_s[j][:, cs],
                                    lhsT=kt_sb[:, j * 128:(j + 1) * 128],
                                    rhs=qt_sb[:, h, qs_], start=True, stop=True)

                            qk(0)
                            if njt > 1:
                                qk(1)
                            for j in range(njt):
                                r = max(0, j - 4 * c)
                                cs = slice(128 * r, CH)
                                p_sb[j] = pp.tile([128, CH], BF16, name="p_sb")
                                nc.scalar.activation(p_sb[j][:, cs],
                                                     ps_s[j][:, cs],
                                                     Exp, scale=SCALE,
                                                     bias=kb_sb[:, j:j + 1])
                                del ps_s[j]
                                if j >= 4 * c:
                                    # triangular mask on the 128-wide diagonal
                                    # block; masks_sb is col >= p
                                    nc.vector.tensor_mul(
                                        p_sb[j][:, 128 * r:128 * (r + 1)],
                                        p_sb[j][:, 128 * r:128 * (r + 1)],
                                        masks_sb)
                                if j + 2 < njt:
                                    qk(j + 2)
                                st, sp = (j == 0), (j == njt - 1)
                                nc.tensor.matmul(ps_pv[:, cs],
                                                 lhsT=v_sb[:, j, :],
                                                 rhs=p_sb[j][:, cs],
                                                 start=st, stop=sp)
                                if j == 0:
                                    eng.tensor_scalar_mul(sum_p, p_sb[j], 1.0)
                                else:
                                    eng.tensor_add(sum_p[:, cs], sum_p[:, cs],
                                                   p_sb[j][:, cs])
                                del p_sb[j]
                            eng.tensor_scalar_mul(sum_pb, sum_p, 1.0)
                            ps_l = ps_l_pool.tile([1, CH], F32, name="ps_l")
                            nc.tensor.matmul(ps_l, lhsT=ones_sb, rhs=sum_pb,
                                             start=True, stop=True)
                            rl = rbp.tile([1, CH], F32, name="rl")
                            nc.vector.reciprocal(rl, ps_l)
                            rb = rbp.tile([128, CH], F32, name="rb")
                            nc.gpsimd.partition_broadcast(rb, rl, channels=128)
                            at_sb = att.tile([128, CH], F16, name="at_sb")
                            nc.vector.tensor_mul(at_sb, ps_pv, rb)
                            nc.sync.dma_start(out=ag_in[h], in_=at_sb)

                        ag_out = dram.tile([G, HPG, 128, CH], F16, name="ag_out")
                        if sim_mode:
                            # stand-in for the AllGather with equivalent local IO
                            for g in range(G):
                                nc.sync.dma_start(out=ag_out[g], in_=ag_in[:])
                        else:
                            nc.gpsimd.collective_compute(
                                "AllGather", mybir.AluOpType.bypass,
                                replica_groups=groups,
                                ins=[ag_in.opt()], outs=[ag_out.opt()],
                            )
                        pending.append((c, ag_out))
                        if c >= OPROJ_LAG:
                            oproj(*pending.pop(0))
                    for item in pending:
                        oproj(*item)

    nc.compile()
    return nc


def _host_consts():
    import ml_dtypes
    bf16 = ml_dtypes.bfloat16
    # causal mask for the 128-wide diagonal key-tile blocks:
    # masks[p, col] = 1.0 iff col >= p   (col = sq offset within the block,
    # p = sk within key tile)
    col = np.arange(128)[None, :]
    p = np.arange(128)[:, None]
    masks = (col >= p).astype(bf16)
    ident = np.eye(128, dtype=bf16)
    ones = np.ones((128, 1), dtype=bf16)
    return masks, ident, ones


def kernel(hidden_states, attention_mask, Wq, bq, Wk, bk, Wv, bv, Wo, bo):
    import ml_dtypes
    from concourse.bass_utils import run_bass_kernel_spmd

    bf16 = ml_dtypes.bfloat16

    global _CACHED_NC
    if _CACHED_NC is None:
        _CACHED_NC = _build_nc()
    nc = _CACHED_NC

    X = np.asarray(hidden_states, dtype=np.float32)
    am = np.asarray(attention_mask).astype(np.float32)  # [B, S] key mask
    Wq = np.asarray(Wq, np.float32)
    Wk = np.asarray(Wk, np.float32)
    Wv = np.asarray(Wv, np.float32)
    Wo = np.asarray(Wo, np.float32)
    masks, ident, ones = _host_consts()

    in_maps = []
    for c in range(8):
        b, g = divmod(c, G)
        qs = slice(g * HPG * D, (g + 1) * HPG * D)   # q-head cols of group g
        ks = slice(g * D, (g + 1) * D)               # kv-head cols of group g
        in_maps.append({
            "xt": np.ascontiguousarray(X[b].T).astype(bf16),
            "wq": np.ascontiguousarray(Wq[:, qs]).astype(bf16),
            "wk": np.ascontiguousarray(Wk[:, ks]).astype(bf16),
            "wv": np.ascontiguousarray(Wv[:, ks]).astype(bf16),
            "wo": np.ascontiguousarray(Wo[:, qs]).astype(np.float16),
            "bq": np.ascontiguousarray(
                np.asarray(bq, np.float32)[qs].reshape(HPG, D).T),
            "bk": np.asarray(bk, np.float32)[ks].reshape(D, 1).copy(),
            "bv": np.asarray(bv, np.float32)[ks].reshape(D, 1).copy(),
            "bo": np.ascontiguousarray(
                np.asarray(bo, np.float32)[qs].reshape(HPG, D).T),
            "masks": masks.copy(),
            "keybias": np.ascontiguousarray(
                ((1.0 - am[b]) * -10000.0).astype(np.float32)
                .reshape(NKT, 128).T),
            "ident": ident.copy(),
            "ones": ones.copy(),
        })

    global _last_in_maps
    _last_in_maps = in_maps
    res = run_bass_kernel_spmd(nc, in_maps, core_ids=list(range(8)))
    out = np.empty((B, S, HID), dtype=np.float32)
    for c in range(8):
        b, g = divmod(c, G)
        out[b][:, g * CH:(g + 1) * CH] = res.results[c]["out"].T
    return out


# revision 4
# speedup vs baseline: 1.4537x; 1.4537x over previous
"""Causal GQA self-attention on 8 Trainium2 NeuronCores.

Problem: B=2, S=2048, HIDDEN=2048, 16 q-heads, 4 kv-heads, head_dim=128, fp32.

Sharding: core c = 4*b + g  (b = batch, g = head-group).
Each core owns batch b and q-heads [4g, 4g+4) plus their shared kv-head g
(GQA maps q-head h -> kv-head h//4, so group g exactly owns kv-head g).

Per-core pipeline (everything in transposed [feature, seq] layout, bf16
matmul operands / fp32 PSUM accumulation):
  1. Projections: QT[d,s] / KT[d,s] / VT[d,s] = W*.T @ X.T  (X.T shipped from
     host pre-cast to bf16).  V[s,d] obtained by PE-transposing VT.
  2. Attention per 512-wide query chunk, per head: for each 128-wide key tile j,
     ST = KT_j.T @ QT_chunk  ->  P = exp(ST/sqrt(d)) (ACT, fused scale)
     -> causal mask multiply on diagonal tiles (DVE)
     -> attnT += V_j.T @ P (PE, PSUM accum).
     Row-sums l: P tiles are accumulated into sumP on the DVE (fp32,
     off the PE critical path) and a single ones.T @ sumP matmul per
     (chunk, head) yields l.  The QK matmul for key tile j+2 is emitted
     before the PV matmul of tile j so the PE never waits on the ACT exp,
     and each head's epilogue (l matmul / normalize) is deferred until
     after the next head's first two QK matmuls.
  3. AllGather attnT chunk across the 4 cores of the batch (f16) -> full
     [2048, 512] attnT; O-projection with this core's 512 Wo columns ->
     outT slice.  Chunks are processed in descending size order (3,2,1,0)
     with the O-projection of the previous chunk emitted after the next
     attention chunk, so AllGather latency hides under attention compute
     and the exposed tail is the cheap chunk-0 gather.
Host gathers: out[b][:, 512g:512(g+1)] = core(b,g) outT.T.

bf16 operands keep the PE at 1 cycle/column and enable Fast Weight Load
(fp32 stationaries disable FWL and run the array in its max-power mode,
which triggers sustained HAM/SW clock throttling).
"""

import numpy as np

HID = 2048
S = 2048
B = 2
NH = 16          # q heads total
D = 128          # head dim
G = 4            # head groups == cores per batch
HPG = NH // G    # q heads per group (4)
CH = 512         # seq chunk (free dim of moving operands)
NCH = S // CH    # 4 chunks
NKT = S // 128   # 16 key tiles
SCALE = 1.0 / float(np.sqrt(D))

_CACHED_NC = None


def _build_nc(sim_mode=False, reps=1):
    import concourse.mybir as mybir
    import concourse.tile as tile
    from concourse import bacc

    F32 = mybir.dt.float32
    F32R = mybir.dt.float32r
    BF16 = mybir.dt.bfloat16
    F16 = mybir.dt.float16
    Copy = mybir.ActivationFunctionType.Copy
    Exp = mybir.ActivationFunctionType.Exp

    nc = bacc.Bacc("TRN2", target_bir_lowering=False, debug=False,
                   num_devices=1 if sim_mode else 8)

    # ---- per-core input shards ----
    xt = nc.declare_dram_parameter("xt", [HID, S], BF16, isOutput=False)
    wq = nc.declare_dram_parameter("wq", [HID, HPG * D], BF16, isOutput=False)
    wk = nc.declare_dram_parameter("wk", [HID, D], BF16, isOutput=False)
    wv = nc.declare_dram_parameter("wv", [HID, D], BF16, isOutput=False)
    wo = nc.declare_dram_parameter("wo", [HID, CH], F16, isOutput=False)
    bq = nc.declare_dram_parameter("bq", [D, HPG], F32, isOutput=False)
    bk = nc.declare_dram_parameter("bk", [D, 1], F32, isOutput=False)
    bv = nc.declare_dram_parameter("bv", [D, 1], F32, isOutput=False)
    bo = nc.declare_dram_parameter("bo", [D, HPG], F32, isOutput=False)
    masks = nc.declare_dram_parameter("masks", [128, 128], BF16, isOutput=False)
    keybias = nc.declare_dram_parameter("keybias", [128, NKT], F32, isOutput=False)
    ident = nc.declare_dram_parameter("ident", [128, 128], BF16, isOutput=False)
    ones = nc.declare_dram_parameter("ones", [128, 1], F32, isOutput=False)
    out = nc.declare_dram_parameter("out", [CH, S], F32, isOutput=True)

    groups = [[0, 1, 2, 3], [4, 5, 6, 7]]

    with tile.TileContext(nc) as tc:
        for _rep in range(reps):
            with (
                tc.tile_pool(name="persist", bufs=1) as persist,
                tc.tile_pool(name="dram", bufs=4, space="DRAM") as dram,
            ):
                # ---- persistent SBUF state ----
                qt_sb = persist.tile([128, HPG, S], BF16)      # QT per head  [d, h, s]
                kt_sb = persist.tile([128, S], BF16)           # KT           [d, s]
                v_sb = persist.tile([128, NKT, D], BF16)       # V            [s, j, d]
                masks_sb = persist.tile([128, 128], BF16)
                kb_sb = persist.tile([128, NKT], F32)
                ones_sb = persist.tile([128, 1], F32R)
                bq_sb = persist.tile([D, HPG], F32)
                bk_sb = persist.tile([D, 1], F32)
                bv_sb = persist.tile([D, 1], F32)
                bo_sb = persist.tile([D, HPG], F32)
                wo_sb = persist.tile([128, NKT, CH], F16)

                nc.sync.dma_start(out=masks_sb, in_=masks.ap())
                nc.sync.dma_start(out=kb_sb, in_=keybias.ap())
                nc.gpsimd.dma_start(out=ones_sb, in_=ones.ap())
                nc.sync.dma_start(out=bq_sb, in_=bq.ap())
                nc.sync.dma_start(out=bk_sb, in_=bk.ap())
                nc.sync.dma_start(out=bv_sb, in_=bv.ap())
                nc.sync.dma_start(out=bo_sb, in_=bo.ap())

                # ================= Phase A: projections =================
                with (
                    tc.tile_pool(name="wA", bufs=1) as wA,
                    tc.tile_pool(name="xs", bufs=6) as xs,
                    tc.tile_pool(name="vts", bufs=2) as vts,
                    tc.tile_pool(name="psA", bufs=1, space="PSUM") as psA,
                    tc.tile_pool(name="psT", bufs=1, space="PSUM") as psT,
                ):
                    idr_sb = wA.tile([128, 128], BF16)
                    nc.gpsimd.dma_start(out=idr_sb, in_=ident.ap())
                    wq_sb = wA.tile([128, NKT, HPG * D], BF16)
                    wq_r = wq.ap().rearrange("(t p) n -> p t n", p=128)
                    nc.gpsimd.dma_start(out=wq_sb[:, :4, :], in_=wq_r[:, :4, :])
                    wk_sb = wA.tile([128, NKT, D], BF16)
                    nc.gpsimd.dma_start(
                        out=wk_sb, in_=wk.ap().rearrange("(t p) n -> p t n", p=128))
                    wv_sb = wA.tile([128, NKT, D], BF16)
                    nc.gpsimd.dma_start(
                        out=wv_sb, in_=wv.ap().rearrange("(t p) n -> p t n", p=128))
                    nc.gpsimd.dma_start(out=wq_sb[:, 4:8, :], in_=wq_r[:, 4:8, :])
                    nc.gpsimd.dma_start(out=wq_sb[:, 8:12, :], in_=wq_r[:, 8:12, :])
                    nc.gpsimd.dma_start(out=wq_sb[:, 12:, :], in_=wq_r[:, 12:, :])

                    for c in range(NCH):
                        sq = slice(c * CH, (c + 1) * CH)
                        ps_q = psA.tile([128, HPG, CH], F32, name="ps_q")  # 4 banks
                        ps_k = psA.tile([128, CH], F32, name="ps_k", bufs=2)
                        ps_v = psA.tile([128, CH], F32, name="ps_v")
                        for t in range(NKT):
                            xt_t = xs.tile([128, CH], BF16, name="xt_t")
                            nc.gpsimd.dma_start(
                                out=xt_t, in_=xt[t * 128:(t + 1) * 128, sq])
                            st, sp = (t == 0), (t == NKT - 1)
                            for h in range(HPG):
                                nc.tensor.matmul(
                                    ps_q[:, h, :],
                                    lhsT=wq_sb[:, t, h * D:(h + 1) * D],
                                    rhs=xt_t, start=st, stop=sp)
                            nc.tensor.matmul(ps_k, lhsT=wk_sb[:, t, :], rhs=xt_t,
                                             start=st, stop=sp)
                            nc.tensor.matmul(ps_v, lhsT=wv_sb[:, t, :], rhs=xt_t,
                                             start=st, stop=sp)
                        for h in range(HPG):
                            nc.vector.tensor_scalar_add(qt_sb[:, h, sq],
                                                        ps_q[:, h, :],
                                                        bq_sb[:, h:h + 1])
                        nc.vector.tensor_scalar_add(kt_sb[:, sq], ps_k, bk_sb)
                        vt_t = vts.tile([128, CH], BF16, name="vt_t")
                        nc.vector.tensor_scalar_add(vt_t, ps_v, bv_sb)
                        for u in range(4):
                            j = 4 * c + u
                            ps_tp = psT.tile([128, 128], BF16, name="ps_tp")
                            nc.tensor.transpose(
                                ps_tp, vt_t[:, u * 128:(u + 1) * 128], idr_sb)
                            nc.scalar.activation(v_sb[:, j, :], ps_tp, Copy)
                        if c == 0:
                            nc.gpsimd.dma_start(
                                out=wo_sb,
                                in_=wo.ap().rearrange("(t p) n -> p t n", p=128))

                # ================= Phase B: attention + O-projection =================
                with (
                    tc.tile_pool(name="ps_s", bufs=3, space="PSUM") as ps_s_pool,
                    tc.tile_pool(name="ps_pv", bufs=2, space="PSUM") as ps_pv_pool,
                    tc.tile_pool(name="ps_l", bufs=1, space="PSUM") as ps_l_pool,
                    tc.tile_pool(name="ps_o", bufs=2, space="PSUM") as ps_o_pool,
                    tc.tile_pool(name="ps_p", bufs=4) as pp,
                    tc.tile_pool(name="sump", bufs=2) as sump,
                    tc.tile_pool(name="att", bufs=2) as att,
                    tc.tile_pool(name="rbp", bufs=2) as rbp,
                    tc.tile_pool(name="mo", bufs=2) as mo,
                    tc.tile_pool(name="ost", bufs=2) as ost,
                ):
                    def oproj(c, ag_out):
                        sq = slice(c * CH, (c + 1) * CH)
                        m_all = mo.tile([128, NKT, CH], F16, name="m_all")
                        ag_r = ag_out.rearrange("g h p n -> p (g h) n")
                        for ct in range(NKT):
                            nc.sync.dma_start(out=m_all[:, ct, :],
                                              in_=ag_r[:, ct, :])
                        for t in range(HPG):
                            ps_o = ps_o_pool.tile([128, CH], F32, name="ps_o")
                            for ct in range(NKT):
                                nc.tensor.matmul(
                                    ps_o, lhsT=wo_sb[:, ct, t * 128:(t + 1) * 128],
                                    rhs=m_all[:, ct, :],
                                    start=(ct == 0), stop=(ct == NKT - 1))
                            o_sb = ost.tile([128, CH], F32, name="o_sb")
                            nc.vector.tensor_scalar_add(o_sb, ps_o,
                                                        bo_sb[:, t:t + 1])
                            nc.sync.dma_start(
                                out=out[t * 128:(t + 1) * 128, sq], in_=o_sb)

                    pending = None
                    for c in [3, 2, 1, 0]:
                        njt = 4 * c + 4  # causal: key tiles 0..4c+3
                        ag_in = dram.tile([HPG, 128, CH], F16, name="ag_in")
                        epilogue = None
                        for h in range(HPG):
                            ps_pv = ps_pv_pool.tile([128, CH], F32, name="ps_pv")
                            sum_p = sump.tile([128, CH], F32R, name="sum_p")
                            ps_s = {}
                            p_sb = {}

                            def qk(j, h=h):
                                # causal: key tile j only reaches queries
                                # >= 128*r into the chunk (r = j - 4c >= 0 on
                                # the diagonal); restrict to that suffix.
                                r = max(0, j - 4 * c)
                                cs = slice(128 * r, CH)
                                qs_ = slice(c * CH + 128 * r, (c + 1) * CH)
                                ps_s[j] = ps_s_pool.tile([128, CH], F32,
                                                         name="ps_s")
                                nc.tensor.matmul(
                                    ps_s[j][:, cs],
                                    lhsT=kt_sb[:, j * 128:(j + 1) * 128],
                                    rhs=qt_sb[:, h, qs_], start=True, stop=True)

                            qk(0)
                            if njt > 1:
                                qk(1)
                            # previous head's l matmul waits on its last DVE
                            # row-sum add; the two QK matmuls above cover it.
                            if epilogue is not None:
                                epilogue()
                            for j in range(njt):
                                r = max(0, j - 4 * c)
                                cs = slice(128 * r, CH)
                                p_sb[j] = pp.tile([128, CH], BF16, name="p_sb")
                                nc.scalar.activation(p_sb[j][:, cs],
                                                     ps_s[j][:, cs],
                                                     Exp, scale=SCALE,
                                                     bias=kb_sb[:, j:j + 1])
                                del ps_s[j]
                                if j >= 4 * c:
                                    # triangular mask on the 128-wide diagonal
                                    # block; masks_sb is col >= p
                                    nc.vector.tensor_mul(
                                        p_sb[j][:, 128 * r:128 * (r + 1)],
                                        p_sb[j][:, 128 * r:128 * (r + 1)],
                                        masks_sb)
                                if j + 2 < njt:
                                    qk(j + 2)
                                st, sp = (j == 0), (j == njt - 1)
                                nc.tensor.matmul(ps_pv[:, cs],
                                                 lhsT=v_sb[:, j, :],
                                                 rhs=p_sb[j][:, cs],
                                                 start=st, stop=sp)
                                if j == 0:
                                    nc.vector.tensor_copy(sum_p, p_sb[j])
                                else:
                                    nc.vector.tensor_add(sum_p[:, cs],
                                                         sum_p[:, cs],
                                                         p_sb[j][:, cs])
                                del p_sb[j]

                            def epilogue(h=h, ps_pv=ps_pv, sum_p=sum_p):
                                ps_l = ps_l_pool.tile([1, CH], F32, name="ps_l")
                                nc.tensor.matmul(ps_l, lhsT=ones_sb, rhs=sum_p,
                                                 start=True, stop=True)
                                rl = rbp.tile([1, CH], F32, name="rl")
                                nc.vector.reciprocal_approx_fast(out=rl, in_=ps_l)
                                rb = rbp.tile([128, CH], F32, name="rb")
                                nc.gpsimd.partition_broadcast(rb, rl, channels=128)
                                at_sb = att.tile([128, CH], F16, name="at_sb")
                                nc.vector.tensor_mul(at_sb, ps_pv, rb)
                                nc.sync.dma_start(out=ag_in[h], in_=at_sb)
                        epilogue()

                        ag_out = dram.tile([G, HPG, 128, CH], F16, name="ag_out")
                        if sim_mode:
                            # stand-in for the AllGather with equivalent local IO
                            for g in range(G):
                                nc.sync.dma_start(out=ag_out[g], in_=ag_in[:])
                        else:
                            nc.gpsimd.collective_compute(
                                "AllGather", mybir.AluOpType.bypass,
                                replica_groups=groups,
                                ins=[ag_in.opt()], outs=[ag_out.opt()],
                            )
                        if pending is not None:
                            oproj(*pending)
                        pending = (c, ag_out)
                    oproj(*pending)

    nc.compile()
    return nc


def _host_consts():
    import ml_dtypes
    bf16 = ml_dtypes.bfloat16
    # causal mask for the 128-wide diagonal key-tile blocks:
    # masks[p, col] = 1.0 iff col >= p   (col = sq offset within the block,
    # p = sk within key tile)
    col = np.arange(128)[None, :]
    p = np.arange(128)[:, None]
    masks = (col >= p).astype(bf16)
    ident = np.eye(128, dtype=bf16)
    ones = np.ones((128, 1), dtype=np.float32)
    return masks, ident, ones


def kernel(hidden_states, attention_mask, Wq, bq, Wk, bk, Wv, bv, Wo, bo):
    import ml_dtypes
    from concourse.bass_utils import run_bass_kernel_spmd

    bf16 = ml_dtypes.bfloat16

    global _CACHED_NC
    if _CACHED_NC is None:
        _CACHED_NC = _build_nc()
    nc = _CACHED_NC

    X = np.asarray(hidden_states, dtype=np.float32)
    am = np.asarray(attention_mask).astype(np.float32)  # [B, S] key mask
    Wq = np.asarray(Wq, np.float32)
    Wk = np.asarray(Wk, np.float32)
    Wv = np.asarray(Wv, np.float32)
    Wo = np.asarray(Wo, np.float32)
    masks, ident, ones = _host_consts()

    in_maps = []
    for c in range(8):
        b, g = divmod(c, G)
        qs = slice(g * HPG * D, (g + 1) * HPG * D)   # q-head cols of group g
        ks = slice(g * D, (g + 1) * D)               # kv-head cols of group g
        in_maps.append({
            "xt": np.ascontiguousarray(X[b].T).astype(bf16),
            "wq": np.ascontiguousarray(Wq[:, qs]).astype(bf16),
            "wk": np.ascontiguousarray(Wk[:, ks]).astype(bf16),
            "wv": np.ascontiguousarray(Wv[:, ks]).astype(bf16),
            "wo": np.ascontiguousarray(Wo[:, qs]).astype(np.float16),
            "bq": np.ascontiguousarray(
                np.asarray(bq, np.float32)[qs].reshape(HPG, D).T),
            "bk": np.asarray(bk, np.float32)[ks].reshape(D, 1).copy(),
            "bv": np.asarray(bv, np.float32)[ks].reshape(D, 1).copy(),
            "bo": np.ascontiguousarray(
                np.asarray(bo, np.float32)[qs].reshape(HPG, D).T),
            "masks": masks.copy(),
            "keybias": np.ascontiguousarray(
                ((1.0 - am[b]) * -10000.0).astype(np.float32)
                .reshape(NKT, 128).T),
            "ident": ident.copy(),
            "ones": ones.copy(),
        })

    global _last_in_maps
    _last_in_maps = in_maps
    res = run_bass_kernel_spmd(nc, in_maps, core_ids=list(range(8)))
    out = np.empty((B, S, HID), dtype=np.float32)
    for c in range(8):
        b, g = divmod(c, G)
        out[b][:, g * CH:(g + 1) * CH] = res.results[c]["out"].T
    return out


# revision 7
# speedup vs baseline: 1.5347x; 1.0557x over previous
"""Causal GQA self-attention on 8 Trainium2 NeuronCores.

Problem: B=2, S=2048, HIDDEN=2048, 16 q-heads, 4 kv-heads, head_dim=128, fp32.

Sharding: core c = 4*b + g  (b = batch, g = head-group).
Each core owns batch b and q-heads [4g, 4g+4) plus their shared kv-head g
(GQA maps q-head h -> kv-head h//4, so group g exactly owns kv-head g).

Per-core pipeline (everything in transposed [feature, seq] layout, bf16
matmul operands / fp32 PSUM accumulation):
  1. Projections: QT[d,s] / KT[d,s] / VT[d,s] = W*.T @ X.T  (X.T shipped from
     host pre-cast to bf16).  V[s,d] obtained by PE-transposing VT.
  2. Attention per 512-wide query chunk, per head: for each 128-wide key tile j,
     ST = KT_j.T @ QT_chunk  ->  P = exp(ST/sqrt(d)) (ACT, fused scale)
     -> causal mask multiply on diagonal tiles (DVE)
     -> attnT += V_j.T @ P (PE, PSUM accum).
     Row-sums l: P tiles are accumulated into sumP on the DVE (fp32,
     off the PE critical path) and a single ones.T @ sumP matmul per
     (chunk, head) yields l.  The QK matmul for key tile j+2 is emitted
     before the PV matmul of tile j so the PE never waits on the ACT exp,
     and each head's epilogue (l matmul / normalize) is deferred until
     after the next head's first two QK matmuls.
  3. AllGather attnT chunk across the 4 cores of the batch (f16) -> full
     [2048, 512] attnT; O-projection with this core's 512 Wo columns ->
     outT slice.  The O-projection of chunk c is emitted after attention
     of chunk c+2, so each AllGather's ~30us mesh latency hides under the
     following attention chunks.
Host gathers: out[b][:, 512g:512(g+1)] = core(b,g) outT.T.

bf16 operands keep the PE at 1 cycle/column and enable Fast Weight Load
(fp32 stationaries disable FWL and run the array in its max-power mode,
which triggers sustained HAM/SW clock throttling).
"""

import numpy as np

HID = 2048
S = 2048
B = 2
NH = 16          # q heads total
D = 128          # head dim
G = 4            # head groups == cores per batch
HPG = NH // G    # q heads per group (4)
CH = 512         # seq chunk (free dim of moving operands)
NCH = S // CH    # 4 chunks
NKT = S // 128   # 16 key tiles
SCALE = 1.0 / float(np.sqrt(D))

_CACHED_NC = None


def _build_nc(sim_mode=False, reps=1):
    import concourse.mybir as mybir
    import concourse.tile as tile
    from concourse import bacc

    F32 = mybir.dt.float32
    F32R = mybir.dt.float32r
    BF16 = mybir.dt.bfloat16
    F16 = mybir.dt.float16
    Copy = mybir.ActivationFunctionType.Copy
    Exp = mybir.ActivationFunctionType.Exp

    nc = bacc.Bacc("TRN2", target_bir_lowering=False, debug=False,
                   num_devices=1 if sim_mode else 8)

    # ---- per-core input shards ----
    xt = nc.declare_dram_parameter("xt", [HID, S], BF16, isOutput=False)
    wq = nc.declare_dram_parameter("wq", [HID, HPG * D], BF16, isOutput=False)
    wk = nc.declare_dram_parameter("wk", [HID, D], BF16, isOutput=False)
    wv = nc.declare_dram_parameter("wv", [HID, D], BF16, isOutput=False)
    wo = nc.declare_dram_parameter("wo", [HID, CH], F16, isOutput=False)
    bq = nc.declare_dram_parameter("bq", [D, HPG], F32, isOutput=False)
    bk = nc.declare_dram_parameter("bk", [D, 1], F32, isOutput=False)
    bv = nc.declare_dram_parameter("bv", [D, 1], F32, isOutput=False)
    bo = nc.declare_dram_parameter("bo", [D, HPG], F32, isOutput=False)
    masks = nc.declare_dram_parameter("masks", [128, 128], BF16, isOutput=False)
    keybias = nc.declare_dram_parameter("keybias", [128, NKT], F32, isOutput=False)
    ident = nc.declare_dram_parameter("ident", [128, 128], BF16, isOutput=False)
    ones = nc.declare_dram_parameter("ones", [128, 1], F32, isOutput=False)
    out = nc.declare_dram_parameter("out", [CH, S], F32, isOutput=True)

    groups = [[0, 1, 2, 3], [4, 5, 6, 7]]

    with tile.TileContext(nc) as tc:
        for _rep in range(reps):
            with (
                tc.tile_pool(name="persist", bufs=1) as persist,
                tc.tile_pool(name="dram", bufs=4, space="DRAM") as dram,
            ):
                # ---- persistent SBUF state ----
                qt_sb = persist.tile([128, HPG, S], BF16)      # QT per head  [d, h, s]
                kt_sb = persist.tile([128, S], BF16)           # KT           [d, s]
                v_sb = persist.tile([128, NKT, D], BF16)       # V            [s, j, d]
                masks_sb = persist.tile([128, 128], BF16)
                kb_sb = persist.tile([128, NKT], F32)
                ones_sb = persist.tile([128, 1], F32R)
                bq_sb = persist.tile([D, HPG], F32)
                bk_sb = persist.tile([D, 1], F32)
                bv_sb = persist.tile([D, 1], F32)
                bo_sb = persist.tile([D, HPG], F32)
                wo_sb = persist.tile([128, NKT, CH], F16)

                nc.sync.dma_start(out=masks_sb, in_=masks.ap())
                nc.sync.dma_start(out=kb_sb, in_=keybias.ap())
                nc.gpsimd.dma_start(out=ones_sb, in_=ones.ap())
                nc.sync.dma_start(out=bq_sb, in_=bq.ap())
                nc.sync.dma_start(out=bk_sb, in_=bk.ap())
                nc.sync.dma_start(out=bv_sb, in_=bv.ap())
                nc.sync.dma_start(out=bo_sb, in_=bo.ap())

                # ================= Phase A: projections =================
                with (
                    tc.tile_pool(name="wA", bufs=1) as wA,
                    tc.tile_pool(name="xs", bufs=6) as xs,
                    tc.tile_pool(name="vts", bufs=2) as vts,
                    tc.tile_pool(name="psA", bufs=1, space="PSUM") as psA,
                    tc.tile_pool(name="psT", bufs=1, space="PSUM") as psT,
                ):
                    idr_sb = wA.tile([128, 128], BF16)
                    nc.gpsimd.dma_start(out=idr_sb, in_=ident.ap())
                    wq_sb = wA.tile([128, NKT, HPG * D], BF16)
                    wq_r = wq.ap().rearrange("(t p) n -> p t n", p=128)
                    nc.gpsimd.dma_start(out=wq_sb[:, :4, :], in_=wq_r[:, :4, :])
                    wk_sb = wA.tile([128, NKT, D], BF16)
                    nc.gpsimd.dma_start(
                        out=wk_sb, in_=wk.ap().rearrange("(t p) n -> p t n", p=128))
                    wv_sb = wA.tile([128, NKT, D], BF16)
                    nc.gpsimd.dma_start(
                        out=wv_sb, in_=wv.ap().rearrange("(t p) n -> p t n", p=128))
                    nc.gpsimd.dma_start(out=wq_sb[:, 4:8, :], in_=wq_r[:, 4:8, :])
                    nc.gpsimd.dma_start(out=wq_sb[:, 8:12, :], in_=wq_r[:, 8:12, :])
                    nc.gpsimd.dma_start(out=wq_sb[:, 12:, :], in_=wq_r[:, 12:, :])

                    for c in range(NCH):
                        sq = slice(c * CH, (c + 1) * CH)
                        ps_q = psA.tile([128, HPG, CH], F32, name="ps_q")  # 4 banks
                        ps_k = psA.tile([128, CH], F32, name="ps_k", bufs=2)
                        ps_v = psA.tile([128, CH], F32, name="ps_v")
                        for t in range(NKT):
                            xt_t = xs.tile([128, CH], BF16, name="xt_t")
                            nc.gpsimd.dma_start(
                                out=xt_t, in_=xt[t * 128:(t + 1) * 128, sq])
                            st, sp = (t == 0), (t == NKT - 1)
                            for h in range(HPG):
                                nc.tensor.matmul(
                                    ps_q[:, h, :],
                                    lhsT=wq_sb[:, t, h * D:(h + 1) * D],
                                    rhs=xt_t, start=st, stop=sp)
                            nc.tensor.matmul(ps_k, lhsT=wk_sb[:, t, :], rhs=xt_t,
                                             start=st, stop=sp)
                            nc.tensor.matmul(ps_v, lhsT=wv_sb[:, t, :], rhs=xt_t,
                                             start=st, stop=sp)
                        for h in range(HPG):
                            nc.vector.tensor_scalar_add(qt_sb[:, h, sq],
                                                        ps_q[:, h, :],
                                                        bq_sb[:, h:h + 1])
                        nc.vector.tensor_scalar_add(kt_sb[:, sq], ps_k, bk_sb)
                        vt_t = vts.tile([128, CH], BF16, name="vt_t")
                        nc.vector.tensor_scalar_add(vt_t, ps_v, bv_sb)
                        for u in range(4):
                            j = 4 * c + u
                            ps_tp = psT.tile([128, 128], BF16, name="ps_tp")
                            nc.tensor.transpose(
                                ps_tp, vt_t[:, u * 128:(u + 1) * 128], idr_sb)
                            nc.scalar.activation(v_sb[:, j, :], ps_tp, Copy)
                        if c == 0:
                            nc.gpsimd.dma_start(
                                out=wo_sb,
                                in_=wo.ap().rearrange("(t p) n -> p t n", p=128))

                # ================= Phase B: attention + O-projection =================
                with (
                    tc.tile_pool(name="ps_s", bufs=3, space="PSUM") as ps_s_pool,
                    tc.tile_pool(name="ps_pv", bufs=2, space="PSUM") as ps_pv_pool,
                    tc.tile_pool(name="ps_l", bufs=1, space="PSUM") as ps_l_pool,
                    tc.tile_pool(name="ps_o", bufs=2, space="PSUM") as ps_o_pool,
                    tc.tile_pool(name="ps_p", bufs=4) as pp,
                    tc.tile_pool(name="sump", bufs=2) as sump,
                    tc.tile_pool(name="att", bufs=2) as att,
                    tc.tile_pool(name="rbp", bufs=2) as rbp,
                    tc.tile_pool(name="mo", bufs=2) as mo,
                    tc.tile_pool(name="ost", bufs=2) as ost,
                ):
                    def oproj(c, ag_out):
                        sq = slice(c * CH, (c + 1) * CH)
                        m_all = mo.tile([128, NKT, CH], F16, name="m_all")
                        ag_r = ag_out.rearrange("g h p n -> p (g h) n")
                        for ct in range(NKT):
                            nc.sync.dma_start(out=m_all[:, ct, :],
                                              in_=ag_r[:, ct, :])
                        for t in range(HPG):
                            ps_o = ps_o_pool.tile([128, CH], F32, name="ps_o")
                            for ct in range(NKT):
                                nc.tensor.matmul(
                                    ps_o, lhsT=wo_sb[:, ct, t * 128:(t + 1) * 128],
                                    rhs=m_all[:, ct, :],
                                    start=(ct == 0), stop=(ct == NKT - 1))
                            o_sb = ost.tile([128, CH], F32, name="o_sb")
                            nc.vector.tensor_scalar_add(o_sb, ps_o,
                                                        bo_sb[:, t:t + 1])
                            nc.sync.dma_start(
                                out=out[t * 128:(t + 1) * 128, sq], in_=o_sb)

                    pending = []
                    for c in [0, 1, 2, 3]:
                        njt = 4 * c + 4  # causal: key tiles 0..4c+3
                        ag_in = dram.tile([HPG, 128, CH], F16, name="ag_in")
                        epilogue = None
                        for h in range(HPG):
                            ps_pv = ps_pv_pool.tile([128, CH], F32, name="ps_pv")
                            sum_p = sump.tile([128, CH], F32R, name="sum_p")
                            ps_s = {}
                            p_sb = {}

                            def qk(j, h=h):
                                # causal: key tile j only reaches queries
                                # >= 128*r into the chunk (r = j - 4c >= 0 on
                                # the diagonal); restrict to that suffix.
                                r = max(0, j - 4 * c)
                                cs = slice(128 * r, CH)
                                qs_ = slice(c * CH + 128 * r, (c + 1) * CH)
                                ps_s[j] = ps_s_pool.tile([128, CH], F32,
                                                         name="ps_s")
                                nc.tensor.matmul(
                                    ps_s[j][:, cs],
                                    lhsT=kt_sb[:, j * 128:(j + 1) * 128],
                                    rhs=qt_sb[:, h, qs_], start=True, stop=True)

                            qk(0)
                            if njt > 1:
                                qk(1)
                            # previous head's l matmul waits on its last DVE
                            # row-sum add; the two QK matmuls above cover it.
                            if epilogue is not None:
                                epilogue()
                            for j in range(njt):
                                r = max(0, j - 4 * c)
                                cs = slice(128 * r, CH)
                                p_sb[j] = pp.tile([128, CH], BF16, name="p_sb")
                                nc.scalar.activation(p_sb[j][:, cs],
                                                     ps_s[j][:, cs],
                                                     Exp, scale=SCALE,
                                                     bias=kb_sb[:, j:j + 1])
                                del ps_s[j]
                                if j >= 4 * c:
                                    # triangular mask on the 128-wide diagonal
                                    # block; masks_sb is col >= p
                                    nc.vector.tensor_mul(
                                        p_sb[j][:, 128 * r:128 * (r + 1)],
                                        p_sb[j][:, 128 * r:128 * (r + 1)],
                                        masks_sb)
                                if j + 2 < njt:
                                    qk(j + 2)
                                st, sp = (j == 0), (j == njt - 1)
                                nc.tensor.matmul(ps_pv[:, cs],
                                                 lhsT=v_sb[:, j, :],
                                                 rhs=p_sb[j][:, cs],
                                                 start=st, stop=sp)
                                if j == 0:
                                    nc.vector.tensor_copy(sum_p, p_sb[j])
                                else:
                                    nc.vector.tensor_add(sum_p[:, cs],
                                                         sum_p[:, cs],
                                                         p_sb[j][:, cs])
                                del p_sb[j]

                            def epilogue(h=h, ps_pv=ps_pv, sum_p=sum_p):
                                ps_l = ps_l_pool.tile([1, CH], F32, name="ps_l")
                                nc.tensor.matmul(ps_l, lhsT=ones_sb, rhs=sum_p,
                                                 start=True, stop=True)
                                rl = rbp.tile([1, CH], F32, name="rl")
                                nc.vector.reciprocal_approx_fast(out=rl, in_=ps_l)
                                rb = rbp.tile([128, CH], F32, name="rb")
                                nc.gpsimd.partition_broadcast(rb, rl, channels=128)
                                at_sb = att.tile([128, CH], F16, name="at_sb")
                                nc.vector.tensor_mul(at_sb, ps_pv, rb)
                                nc.sync.dma_start(out=ag_in[h], in_=at_sb)
                        epilogue()

                        ag_out = dram.tile([G, HPG, 128, CH], F16, name="ag_out")
                        if sim_mode:
                            # stand-in for the AllGather with equivalent local IO
                            for g in range(G):
                                nc.sync.dma_start(out=ag_out[g], in_=ag_in[:])
                        else:
                            nc.gpsimd.collective_compute(
                                "AllGather", mybir.AluOpType.bypass,
                                replica_groups=groups,
                                ins=[ag_in.opt()], outs=[ag_out.opt()],
                            )
                        pending.append((c, ag_out))
                        if c >= 2:
                            oproj(*pending.pop(0))
                    for item in pending:
                        oproj(*item)

    nc.compile()
    return nc


def _host_consts():
    import ml_dtypes
    bf16 = ml_dtypes.bfloat16
    # causal mask for the 128-wide diagonal key-tile blocks:
    # masks[p, col] = 1.0 iff col >= p   (col = sq offset within the block,
    # p = sk within key tile)
    col = np.arange(128)[None, :]
    p = np.arange(128)[:, None]
    masks = (col >= p).astype(bf16)
    ident = np.eye(128, dtype=bf16)
    ones = np.ones((128, 1), dtype=np.float32)
    return masks, ident, ones


def kernel(hidden_states, attention_mask, Wq, bq, Wk, bk, Wv, bv, Wo, bo):
    import ml_dtypes
    from concourse.bass_utils import run_bass_kernel_spmd

    bf16 = ml_dtypes.bfloat16

    global _CACHED_NC
    if _CACHED_NC is None:
        _CACHED_NC = _build_nc()
    nc = _CACHED_NC

    X = np.asarray(hidden_states, dtype=np.float32)
    am = np.asarray(attention_mask).astype(np.float32)  # [B, S] key mask
    Wq = np.asarray(Wq, np.float32)
    Wk = np.asarray(Wk, np.float32)
    Wv = np.asarray(Wv, np.float32)
    Wo = np.asarray(Wo, np.float32)
    masks, ident, ones = _host_consts()

    in_maps = []
    for c in range(8):
        b, g = divmod(c, G)
        qs = slice(g * HPG * D, (g + 1) * HPG * D)   # q-head cols of group g
        ks = slice(g * D, (g + 1) * D)               # kv-head cols of group g
        in_maps.append({
            "xt": np.ascontiguousarray(X[b].T).astype(bf16),
            "wq": np.ascontiguousarray(Wq[:, qs]).astype(bf16),
            "wk": np.ascontiguousarray(Wk[:, ks]).astype(bf16),
            "wv": np.ascontiguousarray(Wv[:, ks]).astype(bf16),
            "wo": np.ascontiguousarray(Wo[:, qs]).astype(np.float16),
            "bq": np.ascontiguousarray(
                np.asarray(bq, np.float32)[qs].reshape(HPG, D).T),
            "bk": np.asarray(bk, np.float32)[ks].reshape(D, 1).copy(),
            "bv": np.asarray(bv, np.float32)[ks].reshape(D, 1).copy(),
            "bo": np.ascontiguousarray(
                np.asarray(bo, np.float32)[qs].reshape(HPG, D).T),
            "masks": masks.copy(),
            "keybias": np.ascontiguousarray(
                ((1.0 - am[b]) * -10000.0).astype(np.float32)
                .reshape(NKT, 128).T),
            "ident": ident.copy(),
            "ones": ones.copy(),
        })

    global _last_in_maps
    _last_in_maps = in_maps
    res = run_bass_kernel_spmd(nc, in_maps, core_ids=list(range(8)))
    out = np.empty((B, S, HID), dtype=np.float32)
    for c in range(8):
        b, g = divmod(c, G)
        out[b][:, g * CH:(g + 1) * CH] = res.results[c]["out"].T
    return out


# revision 9
# speedup vs baseline: 1.5361x; 1.0009x over previous
"""Causal GQA self-attention on 8 Trainium2 NeuronCores.

Problem: B=2, S=2048, HIDDEN=2048, 16 q-heads, 4 kv-heads, head_dim=128, fp32.

Sharding: core c = 4*b + g  (b = batch, g = head-group).
Each core owns batch b and q-heads [4g, 4g+4) plus their shared kv-head g
(GQA maps q-head h -> kv-head h//4, so group g exactly owns kv-head g).

Per-core pipeline (everything in transposed [feature, seq] layout, bf16
matmul operands / fp32 PSUM accumulation):
  1. Projections: QT[d,s] / KT[d,s] / VT[d,s] = W*.T @ X.T  (X.T shipped from
     host pre-cast to bf16).  V[s,d] obtained by PE-transposing VT.
  2. Attention per 512-wide query chunk, per head: for each 128-wide key tile j,
     ST = KT_j.T @ QT_chunk  ->  P = exp(ST/sqrt(d)) (ACT, fused scale)
     -> causal mask multiply on diagonal tiles (DVE)
     -> attnT += V_j.T @ P (PE, PSUM accum).
     Row-sums l: P tiles are accumulated into sumP on the DVE (fp32,
     off the PE critical path) and a single ones.T @ sumP matmul per
     (chunk, head) yields l.  The QK matmul for key tile j+2 is emitted
     before the PV matmul of tile j so the PE never waits on the ACT exp,
     and each head's epilogue (l matmul / normalize) is deferred until
     after the next head's first two QK matmuls.
  3. AllGather attnT chunk across the 4 cores of the batch (f16) -> full
     [2048, 512] attnT; O-projection with this core's 512 Wo columns ->
     outT slice.  The O-projection of chunk c is emitted after attention
     of chunk c+2, so each AllGather's ~30us mesh latency hides under the
     following attention chunks.
Host gathers: out[b][:, 512g:512(g+1)] = core(b,g) outT.T.

bf16 operands keep the PE at 1 cycle/column and enable Fast Weight Load
(fp32 stationaries disable FWL and run the array in its max-power mode,
which triggers sustained HAM/SW clock throttling).
"""

import numpy as np

HID = 2048
S = 2048
B = 2
NH = 16          # q heads total
D = 128          # head dim
G = 4            # head groups == cores per batch
HPG = NH // G    # q heads per group (4)
CH = 512         # seq chunk (free dim of moving operands)
NCH = S // CH    # 4 chunks
NKT = S // 128   # 16 key tiles
SCALE = 1.0 / float(np.sqrt(D))

_CACHED_NC = None


def _build_nc(sim_mode=False, reps=1):
    import concourse.mybir as mybir
    import concourse.tile as tile
    from concourse import bacc

    F32 = mybir.dt.float32
    F32R = mybir.dt.float32r
    BF16 = mybir.dt.bfloat16
    F16 = mybir.dt.float16
    Copy = mybir.ActivationFunctionType.Copy
    Exp = mybir.ActivationFunctionType.Exp

    nc = bacc.Bacc("TRN2", target_bir_lowering=False, debug=False,
                   num_devices=1 if sim_mode else 8)

    # ---- per-core input shards ----
    xt = nc.declare_dram_parameter("xt", [HID, S], BF16, isOutput=False)
    wq = nc.declare_dram_parameter("wq", [HID, HPG * D], BF16, isOutput=False)
    wk = nc.declare_dram_parameter("wk", [HID, D], BF16, isOutput=False)
    wv = nc.declare_dram_parameter("wv", [HID, D], BF16, isOutput=False)
    wo = nc.declare_dram_parameter("wo", [HID, CH], F16, isOutput=False)
    bq = nc.declare_dram_parameter("bq", [D, HPG], F32, isOutput=False)
    bk = nc.declare_dram_parameter("bk", [D, 1], F32, isOutput=False)
    bv = nc.declare_dram_parameter("bv", [D, 1], F32, isOutput=False)
    bo = nc.declare_dram_parameter("bo", [D, HPG], F32, isOutput=False)
    masks = nc.declare_dram_parameter("masks", [128, 128], BF16, isOutput=False)
    keybias = nc.declare_dram_parameter("keybias", [128, NKT], F32, isOutput=False)
    ident = nc.declare_dram_parameter("ident", [128, 128], BF16, isOutput=False)
    ones = nc.declare_dram_parameter("ones", [128, 1], F32, isOutput=False)
    out = nc.declare_dram_parameter("out", [CH, S], F32, isOutput=True)

    groups = [[0, 1, 2, 3], [4, 5, 6, 7]]

    with tile.TileContext(nc) as tc:
        for _rep in range(reps):
            with (
                tc.tile_pool(name="persist", bufs=1) as persist,
                tc.tile_pool(name="dram", bufs=4, space="DRAM") as dram,
            ):
                # ---- persistent SBUF state ----
                qt_sb = persist.tile([128, HPG, S], BF16)      # QT per head  [d, h, s]
                kt_sb = persist.tile([128, S], BF16)           # KT           [d, s]
                v_sb = persist.tile([128, NKT, D], BF16)       # V            [s, j, d]
                masks_sb = persist.tile([128, 128], BF16)
                kb_sb = persist.tile([128, NKT], F32)
                ones_sb = persist.tile([128, 1], F32R)
                bq_sb = persist.tile([D, HPG], F32)
                bk_sb = persist.tile([D, 1], F32)
                bv_sb = persist.tile([D, 1], F32)
                bo_sb = persist.tile([D, HPG], F32)
                wo_sb = persist.tile([128, NKT, CH], F16)

                nc.sync.dma_start(out=masks_sb, in_=masks.ap())
                nc.sync.dma_start(out=kb_sb, in_=keybias.ap())
                nc.gpsimd.dma_start(out=ones_sb, in_=ones.ap())
                nc.sync.dma_start(out=bq_sb, in_=bq.ap())
                nc.sync.dma_start(out=bk_sb, in_=bk.ap())
                nc.sync.dma_start(out=bv_sb, in_=bv.ap())
                nc.sync.dma_start(out=bo_sb, in_=bo.ap())

                # tiny prologue AllGather: absorbs the one-time collective
                # barrier + stream setup under Phase A compute, so the first
                # real gather starts without the ~11us warmup delay
                dummy_in = dram.tile([128, 4], F16, name="dummy_in")
                dummy_out = dram.tile([G, 128, 4], F16, name="dummy_out")
                if sim_mode:
                    nc.sync.dma_start(out=dummy_out[0], in_=dummy_in[:])
                else:
                    nc.gpsimd.collective_compute(
                        "AllGather", mybir.AluOpType.bypass,
                        replica_groups=groups,
                        ins=[dummy_in.opt()], outs=[dummy_out.opt()],
                    )

                # ================= Phase A: projections =================
                with (
                    tc.tile_pool(name="wA", bufs=1) as wA,
                    tc.tile_pool(name="xs", bufs=6) as xs,
                    tc.tile_pool(name="vts", bufs=2) as vts,
                    tc.tile_pool(name="psA", bufs=1, space="PSUM") as psA,
                    tc.tile_pool(name="psT", bufs=1, space="PSUM") as psT,
                ):
                    idr_sb = wA.tile([128, 128], BF16)
                    nc.gpsimd.dma_start(out=idr_sb, in_=ident.ap())
                    wq_sb = wA.tile([128, NKT, HPG * D], BF16)
                    wq_r = wq.ap().rearrange("(t p) n -> p t n", p=128)
                    nc.gpsimd.dma_start(out=wq_sb[:, :4, :], in_=wq_r[:, :4, :])
                    wk_sb = wA.tile([128, NKT, D], BF16)
                    nc.gpsimd.dma_start(
                        out=wk_sb, in_=wk.ap().rearrange("(t p) n -> p t n", p=128))
                    wv_sb = wA.tile([128, NKT, D], BF16)
                    nc.gpsimd.dma_start(
                        out=wv_sb, in_=wv.ap().rearrange("(t p) n -> p t n", p=128))
                    nc.gpsimd.dma_start(out=wq_sb[:, 4:8, :], in_=wq_r[:, 4:8, :])
                    nc.gpsimd.dma_start(out=wq_sb[:, 8:12, :], in_=wq_r[:, 8:12, :])
                    nc.gpsimd.dma_start(out=wq_sb[:, 12:, :], in_=wq_r[:, 12:, :])

                    for c in range(NCH):
                        sq = slice(c * CH, (c + 1) * CH)
                        ps_q = psA.tile([128, HPG, CH], F32, name="ps_q")  # 4 banks
                        ps_k = psA.tile([128, CH], F32, name="ps_k", bufs=2)
                        ps_v = psA.tile([128, CH], F32, name="ps_v")
                        for t in range(NKT):
                            xt_t = xs.tile([128, CH], BF16, name="xt_t")
                            nc.gpsimd.dma_start(
                                out=xt_t, in_=xt[t * 128:(t + 1) * 128, sq])
                            st, sp = (t == 0), (t == NKT - 1)
                            for h in range(HPG):
                                nc.tensor.matmul(
                                    ps_q[:, h, :],
                                    lhsT=wq_sb[:, t, h * D:(h + 1) * D],
                                    rhs=xt_t, start=st, stop=sp)
                            nc.tensor.matmul(ps_k, lhsT=wk_sb[:, t, :], rhs=xt_t,
                                             start=st, stop=sp)
                            nc.tensor.matmul(ps_v, lhsT=wv_sb[:, t, :], rhs=xt_t,
                                             start=st, stop=sp)
                        for h in range(HPG):
                            nc.vector.tensor_scalar_add(qt_sb[:, h, sq],
                                                        ps_q[:, h, :],
                                                        bq_sb[:, h:h + 1])
                        nc.vector.tensor_scalar_add(kt_sb[:, sq], ps_k, bk_sb)
                        vt_t = vts.tile([128, CH], BF16, name="vt_t")
                        nc.vector.tensor_scalar_add(vt_t, ps_v, bv_sb)
                        for u in range(4):
                            j = 4 * c + u
                            ps_tp = psT.tile([128, 128], BF16, name="ps_tp")
                            nc.tensor.transpose(
                                ps_tp, vt_t[:, u * 128:(u + 1) * 128], idr_sb)
                            nc.scalar.activation(v_sb[:, j, :], ps_tp, Copy)
                        if c == 0:
                            nc.gpsimd.dma_start(
                                out=wo_sb,
                                in_=wo.ap().rearrange("(t p) n -> p t n", p=128))

                # ================= Phase B: attention + O-projection =================
                with (
                    tc.tile_pool(name="ps_s", bufs=3, space="PSUM") as ps_s_pool,
                    tc.tile_pool(name="ps_pv", bufs=2, space="PSUM") as ps_pv_pool,
                    tc.tile_pool(name="ps_l", bufs=1, space="PSUM") as ps_l_pool,
                    tc.tile_pool(name="ps_o", bufs=2, space="PSUM") as ps_o_pool,
                    tc.tile_pool(name="ps_p", bufs=4) as pp,
                    tc.tile_pool(name="sump", bufs=2) as sump,
                    tc.tile_pool(name="att", bufs=2) as att,
                    tc.tile_pool(name="rbp", bufs=2) as rbp,
                    tc.tile_pool(name="mo", bufs=2) as mo,
                    tc.tile_pool(name="ost", bufs=2) as ost,
                ):
                    def oproj(c, ag_out):
                        sq = slice(c * CH, (c + 1) * CH)
                        m_all = mo.tile([128, NKT, CH], F16, name="m_all")
                        ag_r = ag_out.rearrange("g h p n -> p (g h) n")
                        for ct in range(NKT):
                            nc.sync.dma_start(out=m_all[:, ct, :],
                                              in_=ag_r[:, ct, :])
                        for t in range(HPG):
                            ps_o = ps_o_pool.tile([128, CH], F32, name="ps_o")
                            for ct in range(NKT):
                                nc.tensor.matmul(
                                    ps_o, lhsT=wo_sb[:, ct, t * 128:(t + 1) * 128],
                                    rhs=m_all[:, ct, :],
                                    start=(ct == 0), stop=(ct == NKT - 1))
                            o_sb = ost.tile([128, CH], F32, name="o_sb")
                            nc.vector.tensor_scalar_add(o_sb, ps_o,
                                                        bo_sb[:, t:t + 1])
                            nc.sync.dma_start(
                                out=out[t * 128:(t + 1) * 128, sq], in_=o_sb)

                    state = {"ep": None, "ag": None}
                    gathered = []

                    def issue_ag(c, ag_in):
                        ag_out = dram.tile([G, HPG, 128, CH], F16,
                                           name="ag_out")
                        if sim_mode:
                            # stand-in for the AllGather with equivalent IO
                            for g in range(G):
                                nc.sync.dma_start(out=ag_out[g], in_=ag_in[:])
                        else:
                            nc.gpsimd.collective_compute(
                                "AllGather", mybir.AluOpType.bypass,
                                replica_groups=groups,
                                ins=[ag_in.opt()], outs=[ag_out.opt()],
                            )
                        gathered.append((c, ag_out))

                    def flush():
                        # previous head's epilogue (l matmul waits on its last
                        # DVE row-sum add; the two QK matmuls emitted just
                        # before this cover it), then the previous chunk's
                        # AllGather once its last head's at_sb DMA is queued
                        if state["ep"] is not None:
                            state["ep"]()
                            state["ep"] = None
                        if state["ag"] is not None:
                            issue_ag(*state["ag"])
                            state["ag"] = None

                    for idx, c in enumerate([1, 2, 3, 0]):
                        njt = 4 * c + 4  # causal: key tiles 0..4c+3
                        ag_in = dram.tile([HPG, 128, CH], F16, name="ag_in")
                        for h in range(HPG):
                            ps_pv = ps_pv_pool.tile([128, CH], F32, name="ps_pv")
                            sum_p = sump.tile([128, CH], F32R, name="sum_p")
                            ps_s = {}
                            p_sb = {}

                            def qk(j, h=h):
                                # causal: key tile j only reaches queries
                                # >= 128*r into the chunk (r = j - 4c >= 0 on
                                # the diagonal); restrict to that suffix.
                                r = max(0, j - 4 * c)
                                cs = slice(128 * r, CH)
                                qs_ = slice(c * CH + 128 * r, (c + 1) * CH)
                                ps_s[j] = ps_s_pool.tile([128, CH], F32,
                                                         name="ps_s")
                                nc.tensor.matmul(
                                    ps_s[j][:, cs],
                                    lhsT=kt_sb[:, j * 128:(j + 1) * 128],
                                    rhs=qt_sb[:, h, qs_], start=True, stop=True)

                            qk(0)
                            if njt > 1:
                                qk(1)
                            flush()
                            for j in range(njt):
                                r = max(0, j - 4 * c)
                                cs = slice(128 * r, CH)
                                p_sb[j] = pp.tile([128, CH], BF16, name="p_sb")
                                nc.scalar.activation(p_sb[j][:, cs],
                                                     ps_s[j][:, cs],
                                                     Exp, scale=SCALE,
                                                     bias=kb_sb[:, j:j + 1])
                                del ps_s[j]
                                if j >= 4 * c:
                                    # triangular mask on the 128-wide diagonal
                                    # block; masks_sb is col >= p
                                    nc.vector.tensor_mul(
                                        p_sb[j][:, 128 * r:128 * (r + 1)],
                                        p_sb[j][:, 128 * r:128 * (r + 1)],
                                        masks_sb)
                                if j + 2 < njt:
                                    qk(j + 2)
                                st, sp = (j == 0), (j == njt - 1)
                                nc.tensor.matmul(ps_pv[:, cs],
                                                 lhsT=v_sb[:, j, :],
                                                 rhs=p_sb[j][:, cs],
                                                 start=st, stop=sp)
                                # fold P tiles into the row-sum accumulator on
                                # the DVE; j=0 and j=1 merge into one add when
                                # both are full width (c >= 1)
                                if j == 0 and c >= 1:
                                    pass
                                elif j == 1 and c >= 1:
                                    nc.vector.tensor_add(sum_p, p_sb[0],
                                                         p_sb[1])
                                    del p_sb[0]
                                elif j == 0:
                                    nc.vector.tensor_copy(sum_p, p_sb[j])
                                else:
                                    nc.vector.tensor_add(sum_p[:, cs],
                                                         sum_p[:, cs],
                                                         p_sb[j][:, cs])
                                if j in p_sb and not (j == 0 and c >= 1):
                                    del p_sb[j]

                            def epilogue(h=h, ps_pv=ps_pv, sum_p=sum_p,
                                         ag_in=ag_in):
                                ps_l = ps_l_pool.tile([1, CH], F32, name="ps_l")
                                nc.tensor.matmul(ps_l, lhsT=ones_sb, rhs=sum_p,
                                                 start=True, stop=True)
                                rl = rbp.tile([1, CH], F32, name="rl")
                                nc.vector.reciprocal_approx_fast(out=rl, in_=ps_l)
                                rb = rbp.tile([128, CH], F32, name="rb")
                                nc.gpsimd.partition_broadcast(rb, rl, channels=128)
                                at_sb = att.tile([128, CH], F16, name="at_sb")
                                nc.vector.tensor_mul(at_sb, ps_pv, rb)
                                nc.sync.dma_start(out=ag_in[h], in_=at_sb)
                            state["ep"] = epilogue
                        state["ag"] = (c, ag_in)
                        if idx >= 2:
                            oproj(*gathered.pop(0))
                    flush()
                    for item in gathered:
                        oproj(*item)

    nc.compile()
    return nc


def _host_consts():
    import ml_dtypes
    bf16 = ml_dtypes.bfloat16
    # causal mask for the 128-wide diagonal key-tile blocks:
    # masks[p, col] = 1.0 iff col >= p   (col = sq offset within the block,
    # p = sk within key tile)
    col = np.arange(128)[None, :]
    p = np.arange(128)[:, None]
    masks = (col >= p).astype(bf16)
    ident = np.eye(128, dtype=bf16)
    ones = np.ones((128, 1), dtype=np.float32)
    return masks, ident, ones


def kernel(hidden_states, attention_mask, Wq, bq, Wk, bk, Wv, bv, Wo, bo):
    import ml_dtypes
    from concourse.bass_utils import run_bass_kernel_spmd

    bf16 = ml_dtypes.bfloat16

    global _CACHED_NC
    if _CACHED_NC is None:
        _CACHED_NC = _build_nc()
    nc = _CACHED_NC

    X = np.asarray(hidden_states, dtype=np.float32)
    am = np.asarray(attention_mask).astype(np.float32)  # [B, S] key mask
    Wq = np.asarray(Wq, np.float32)
    Wk = np.asarray(Wk, np.float32)
    Wv = np.asarray(Wv, np.float32)
    Wo = np.asarray(Wo, np.float32)
    masks, ident, ones = _host_consts()

    in_maps = []
    for c in range(8):
        b, g = divmod(c, G)
        qs = slice(g * HPG * D, (g + 1) * HPG * D)   # q-head cols of group g
        ks = slice(g * D, (g + 1) * D)               # kv-head cols of group g
        in_maps.append({
            "xt": np.ascontiguousarray(X[b].T).astype(bf16),
            "wq": np.ascontiguousarray(Wq[:, qs]).astype(bf16),
            "wk": np.ascontiguousarray(Wk[:, ks]).astype(bf16),
            "wv": np.ascontiguousarray(Wv[:, ks]).astype(bf16),
            "wo": np.ascontiguousarray(Wo[:, qs]).astype(np.float16),
            "bq": np.ascontiguousarray(
                np.asarray(bq, np.float32)[qs].reshape(HPG, D).T),
            "bk": np.asarray(bk, np.float32)[ks].reshape(D, 1).copy(),
            "bv": np.asarray(bv, np.float32)[ks].reshape(D, 1).copy(),
            "bo": np.ascontiguousarray(
                np.asarray(bo, np.float32)[qs].reshape(HPG, D).T),
            "masks": masks.copy(),
            "keybias": np.ascontiguousarray(
                ((1.0 - am[b]) * -10000.0).astype(np.float32)
                .reshape(NKT, 128).T),
            "ident": ident.copy(),
            "ones": ones.copy(),
        })

    global _last_in_maps
    _last_in_maps = in_maps
    res = run_bass_kernel_spmd(nc, in_maps, core_ids=list(range(8)))
    out = np.empty((B, S, HID), dtype=np.float32)
    for c in range(8):
        b, g = divmod(c, G)
        out[b][:, g * CH:(g + 1) * CH] = res.results[c]["out"].T
    return out
